# revision 8
# baseline (speedup 1.0000x reference)
"""Multi-head attention (RoPE + QK-RMSnorm + full softmax) on 8 TRN2 NeuronCores.

Sharding: data-parallel over batch (B=8 -> one batch element per core), weights
replicated, no collectives. Per core:

  x[b] : [T=2048, C=384], 6 heads x 64 dim.

Layouts on chip (per core):
  - xT   [C, T]   (host-pre-transposed) so projections contract c on partitions
  - q,k,v projected in natural [t, C] layout (M=t on psum partitions)
  - RoPE + RMS-norm on q,k in natural layout (free-axis reductions)
  - q,k transposed head-wise to qT/kT [C, T] (PE transposes) for attention
  - scores^T [s, t] = kT_h.T @ qT_h  (psum, fp32), exp on ACT -> expS^T bf16
  - PV: y[t-tile, 65] = sum_s expS^T[s, t-tile].T @ [v_h | 1]  (bf16, psum f32)
    column 64 of rhs is ones -> row sums for softmax normalization
  - normalize y by reciprocal row sums (per-partition scalars), transpose y,
    output projection with WoT -> natural [t, C] output
"""

import math
from contextlib import ExitStack

import numpy as np
import ml_dtypes

import concourse.bass as bass
import concourse.mybir as mybir
import concourse.tile as tile
from concourse import bacc
from concourse.bass_utils import run_bass_kernel_spmd
from concourse.masks import make_identity

P = 128
T = 2048
C = 384
H = 6
D = 64
HALF = D // 2  # 32
NT = T // P  # 16 t-tiles
TBW = 512  # t-block width for QK rhs
NTB = T // TBW  # 4
NCT = C // P  # 3 c-tiles
NB = 8  # batch == cores
EPS = 1e-5

F32 = mybir.dt.float32
F32R = mybir.dt.float32r
BF16 = mybir.dt.bfloat16

AF = mybir.ActivationFunctionType
ALU = mybir.AluOpType


def r32(ap):
    return ap.bitcast(F32R)


def build_kernel():
    nc = bacc.Bacc("TRN2", target_bir_lowering=False, debug=False)

    xT_d = nc.dram_tensor("xT", [C, T], F32R, kind="ExternalInput")
    wqT_d = nc.dram_tensor("wqT", [C, C], F32R, kind="ExternalInput")
    wkT_d = nc.dram_tensor("wkT", [C, C], F32R, kind="ExternalInput")
    wvT_d = nc.dram_tensor("wvT", [C, C], F32R, kind="ExternalInput")
    woT_d = nc.dram_tensor("woT", [C, C], BF16, kind="ExternalInput")
    cos_d = nc.dram_tensor("cosF", [T, HALF], F32, kind="ExternalInput")
    sin_d = nc.dram_tensor("sinF", [T, HALF], F32, kind="ExternalInput")
    out_d = nc.dram_tensor("out", [T, C], F32, kind="ExternalOutput")

    with ExitStack() as ctx:
        tc = ctx.enter_context(tile.TileContext(nc))
        res = ctx.enter_context(tc.tile_pool(name="res", bufs=1))
        work = ctx.enter_context(tc.tile_pool(name="work", bufs=3))
        expp = ctx.enter_context(tc.tile_pool(name="expp", bufs=2))
        outp = ctx.enter_context(tc.tile_pool(name="outp", bufs=2))
        # PSUM: big pool holds QK score batches [128,2,512]f32 (2 banks/slot)
        # and projection accumulators [128,384]f32; small pool holds PV
        # accumulators / transposes / out-proj (1 bank/slot). 3*2 + 2 = 8 banks.
        pbig = ctx.enter_context(tc.tile_pool(name="pbig", bufs=2, space="PSUM"))
        psmall = ctx.enter_context(tc.tile_pool(name="psmall", bufs=4, space="PSUM"))

        # ---- resident tiles ----
        xT_sb = res.tile([P, NCT, T], F32R)
        wq_sb = res.tile([P, NCT, C], F32R)
        wk_sb = res.tile([P, NCT, C], F32R)
        wv_sb = res.tile([P, NCT, C], F32R)
        wo_sb = res.tile([P, NCT, C], BF16)
        cos_sb = res.tile([P, NT, HALF], F32)
        sin_sb = res.tile([P, NT, HALF], F32)
        ident = res.tile([P, P], BF16)
        qT_sb = res.tile([P, NCT, T], BF16)
        kT_sb = res.tile([P, NCT, T], BF16)
        v_ext = res.tile([P, NT, H, D + 1], BF16)
        y_sb = res.tile([P, NT, H, D + 1], F32)

        for ci in range(NCT):
            nc.sync.dma_start(
                xT_sb[:, ci, :], xT_d[ci * P : (ci + 1) * P, :]
            )
        for w_sb, w_d in ((wq_sb, wqT_d), (wk_sb, wkT_d), (wv_sb, wvT_d), (wo_sb, woT_d)):
            for ci in range(NCT):
                nc.sync.dma_start(w_sb[:, ci, :], w_d[ci * P : (ci + 1) * P, :])
        nc.sync.dma_start(cos_sb[:], cos_d[:].rearrange("(i p) f -> p i f", p=P))
        nc.sync.dma_start(sin_sb[:], sin_d[:].rearrange("(i p) f -> p i f", p=P))
        make_identity(nc, ident[:])
        # ones column for row sums (col D of every head slot stays 1.0)
        nc.vector.memset(v_ext[:], 1.0)
        eps_sb = res.tile([P, 1], F32)
        nc.vector.memset(eps_sb[:], EPS)

        # ---- stage 1: projections (+ RoPE + RMS-norm for q,k) ----
        def proj_psum(w_sb, i):
            ps = pbig.tile([P, C], F32, tag="pbig")
            for ci in range(NCT):
                nc.tensor.matmul(
                    ps[:],
                    xT_sb[:, ci, i * P : (i + 1) * P],
                    w_sb[:, ci, :],
                    start=(ci == 0),
                    stop=(ci == NCT - 1),
                )
            return ps

        # v first: needed earliest by PV
        for i in range(NT):
            ps = proj_psum(wv_sb, i)
            nc.scalar.copy(
                v_ext[:, i, :, 0:D], ps[:].rearrange("p (h d) -> p h d", h=H)
            )

        def rope_rms_transpose(w_sb, i, dstT):
            ps = proj_psum(w_sb, i)  # [P, C] f32, natural [t, (h d)]
            ph = ps[:].rearrange("p (h d) -> p h d", h=H)
            x1 = ph[:, :, 0:HALF]
            x2 = ph[:, :, HALF:D]
            cb = cos_sb[:, i, :].unsqueeze(1).broadcast_to([P, H, HALF])
            sb = sin_sb[:, i, :].unsqueeze(1).broadcast_to([P, H, HALF])

            rn = work.tile([P, H, D], F32, tag="rope_out")
            t1 = work.tile([P, H, HALF], F32, tag="rope_t1")
            t2 = work.tile([P, H, HALF], F32, tag="rope_t2")
            t3 = work.tile([P, H, HALF], F32, tag="rope_t3")
            t4 = work.tile([P, H, HALF], F32, tag="rope_t4")
            # y1 = x1*cos + x2*sin ; y2 = x2*cos - x1*sin
            nc.vector.tensor_mul(t1[:], x1, cb)
            nc.vector.tensor_mul(t2[:], x2, sb)
            nc.vector.tensor_add(rn[:, :, 0:HALF], t1[:], t2[:])
            nc.vector.tensor_mul(t3[:], x2, cb)
            nc.vector.tensor_mul(t4[:], x1, sb)
            nc.vector.tensor_sub(rn[:, :, HALF:D], t3[:], t4[:])

            # rms scale = 1/sqrt(mean(rn^2) + eps), per (t, h)
            sq = work.tile([P, H, D], F32, tag="sq")
            ms = work.tile([P, H], F32, tag="ms")
            rden = work.tile([P, H], F32, tag="rden")
            rinv = work.tile([P, H], F32, tag="rinv")
            nc.scalar.square(sq[:], rn[:])
            nc.vector.tensor_reduce(ms[:], sq[:], axis=mybir.AxisListType.X, op=ALU.add)
            nc.scalar.activation(rden[:], ms[:], AF.Sqrt, bias=eps_sb[:], scale=1.0 / D)
            nc.vector.reciprocal(rinv[:], rden[:])

            qn = work.tile([P, C], BF16, tag="qn")
            qnh = qn[:].rearrange("p (h d) -> p h d", h=H)
            for h in range(H):
                nc.vector.tensor_scalar_mul(
                    qnh[:, h, :], rn[:, h, :], rinv[:, h : h + 1]
                )
            # transpose [t, c] -> [c, t] per c-tile
            for ci in range(NCT):
                tp = psmall.tile([P, P], BF16, tag="psmall")
                nc.tensor.transpose(tp[:], qn[:, ci * P : (ci + 1) * P], ident[:])
                nc.scalar.copy(dstT[:, ci, i * P : (i + 1) * P], tp[:])

        for i in range(NT):
            rope_rms_transpose(wk_sb, i, kT_sb)
        for i in range(NT):
            rope_rms_transpose(wq_sb, i, qT_sb)

        # ---- stage 2: attention per (head, t-block) ----
        for h in range(H):
            ci = h // 2
            r0 = (h % 2) * D  # partition offset of head h inside c-tile ci
            for tb in range(NTB):
                expS = expp.tile([P, NT, TBW], BF16, tag="expS")
                for j2 in range(NT // 2):
                    sps = pbig.tile([P, 2, TBW], F32, tag="pbig")
                    for jj in range(2):
                        j = j2 * 2 + jj
                        nc.tensor.matmul(
                            sps[:, jj, :],
                            kT_sb[r0 : r0 + D, ci, j * P : (j + 1) * P],
                            qT_sb[r0 : r0 + D, ci, tb * TBW : (tb + 1) * TBW],
                            start=True,
                            stop=True,
                        )
                    nc.scalar.activation(
                        expS[:, 2 * j2 : 2 * j2 + 2, :],
                        sps[:],
                        AF.Exp,
                        scale=1.0 / math.sqrt(D),
                    )
                for ti in range(TBW // P):
                    it = tb * (TBW // P) + ti  # global t-tile
                    yp = psmall.tile([P, D + 1], F32, tag="psmall")
                    for j in range(NT):
                        nc.tensor.matmul(
                            yp[:],
                            expS[:, j, ti * P : (ti + 1) * P],
                            v_ext[:, j, h, :],
                            start=(j == 0),
                            stop=(j == NT - 1),
                        )
                    nc.vector.tensor_copy(y_sb[:, it, h, :], yp[:])

        # ---- stage 3: normalize + transpose + output projection ----
        for i in range(NT):
            rs = work.tile([P, H], F32, tag="rs")
            nc.vector.reciprocal(rs[:], y_sb[:, i, :, D])
            yn = work.tile([P, C], BF16, tag="yn")
            ynh = yn[:].rearrange("p (h d) -> p h d", h=H)
            for h in range(H):
                nc.vector.tensor_scalar_mul(
                    ynh[:, h, :], y_sb[:, i, h, 0:D], rs[:, h : h + 1]
                )
            op = pbig.tile([P, C], F32, tag="pbig")
            for ci in range(NCT):
                tp = psmall.tile([P, P], BF16, tag="psmall")
                nc.tensor.transpose(tp[:], yn[:, ci * P : (ci + 1) * P], ident[:])
                ytc = work.tile([P, P], BF16, tag="ytc")
                nc.scalar.copy(ytc[:], tp[:])
                nc.tensor.matmul(
                    op[:],
                    ytc[:],
                    wo_sb[:, ci, :],
                    start=(ci == 0),
                    stop=(ci == NCT - 1),
                )
            ot = outp.tile([P, C], F32, tag="ot")
            nc.vector.tensor_copy(ot[:], op[:])
            nc.sync.dma_start(out_d[i * P : (i + 1) * P, :], ot[:])

    nc.compile()
    return nc


_NC_CACHE = None


def _get_nc():
    global _NC_CACHE
    if _NC_CACHE is None:
        _NC_CACHE = build_kernel()
    return _NC_CACHE


def _in_maps(x, Wq, Wk, Wv, Wo, cos, sin):
    f32 = np.float32
    shared = {
        "wqT": np.ascontiguousarray(np.asarray(Wq, f32).T),
        "wkT": np.ascontiguousarray(np.asarray(Wk, f32).T),
        "wvT": np.ascontiguousarray(np.asarray(Wv, f32).T),
        "woT": np.ascontiguousarray(
            np.asarray(Wo, f32).T.astype(ml_dtypes.bfloat16)
        ),
        "cosF": np.ascontiguousarray(np.asarray(cos, f32).reshape(T, HALF)),
        "sinF": np.ascontiguousarray(np.asarray(sin, f32).reshape(T, HALF)),
    }
    x = np.asarray(x, f32)
    return [
        {"xT": np.ascontiguousarray(x[b].T), **shared} for b in range(NB)
    ]


def kernel(x, Wq, Wk, Wv, Wo, cos, sin, _profile=False):
    nc = _get_nc()
    in_maps = _in_maps(x, Wq, Wk, Wv, Wo, cos, sin)
    r = run_bass_kernel_spmd(nc, in_maps, core_ids=list(range(NB)), trace=_profile)
    out = np.stack([r.results[b]["out"] for b in range(NB)]).astype(np.float32)
    if _profile:
        kernel.last_results = r
    return out


# revision 15
# speedup vs baseline: 1.0673x; 1.0673x over previous
"""Multi-head attention (RoPE + QK-RMSnorm + full softmax) on 8 TRN2 NeuronCores.

Sharding: data-parallel over batch (B=8 -> one batch element per core), weights
replicated, no collectives. Per core:

  x[b] : [T=2048, C=384], 6 heads x 64 dim.

Layouts on chip (per core):
  - xT   [C, T]   (host-pre-transposed) so projections contract c on partitions
  - q,k,v projected in natural [t, C] layout (M=t on psum partitions)
  - RoPE + RMS-norm on q,k in natural layout (free-axis reductions)
  - q,k transposed head-wise to qT/kT [C, T] (PE transposes) for attention
  - scores^T [s, t] = kT_h.T @ qT_h  (psum, fp32), exp on ACT -> expS^T bf16
  - PV: y[t-tile, 65] = sum_s expS^T[s, t-tile].T @ [v_h | 1]  (bf16, psum f32)
    column 64 of rhs is ones -> row sums for softmax normalization
  - normalize y by reciprocal row sums (per-partition scalars), transpose y,
    output projection with WoT -> natural [t, C] output
"""

import math
from contextlib import ExitStack

import numpy as np
import ml_dtypes

import concourse.bass as bass
import concourse.mybir as mybir
import concourse.tile as tile
from concourse import bacc
from concourse.bass_utils import run_bass_kernel_spmd
from concourse.masks import make_identity

P = 128
T = 2048
C = 384
H = 6
D = 64
HALF = D // 2  # 32
NT = T // P  # 16 t-tiles
TBW = 512  # t-block width for QK rhs
NTB = T // TBW  # 4
NCT = C // P  # 3 c-tiles
NB = 8  # batch == cores
EPS = 1e-5

F32 = mybir.dt.float32
F32R = mybir.dt.float32r
BF16 = mybir.dt.bfloat16

AF = mybir.ActivationFunctionType
ALU = mybir.AluOpType


def r32(ap):
    return ap.bitcast(F32R)


def build_kernel():
    nc = bacc.Bacc("TRN2", target_bir_lowering=False, debug=False)

    xT_d = nc.dram_tensor("xT", [C, T], F32R, kind="ExternalInput")
    wqT_d = nc.dram_tensor("wqT", [C, C], F32R, kind="ExternalInput")
    wkT_d = nc.dram_tensor("wkT", [C, C], F32R, kind="ExternalInput")
    wvT_d = nc.dram_tensor("wvT", [C, C], F32R, kind="ExternalInput")
    woT_d = nc.dram_tensor("woT", [C, C], BF16, kind="ExternalInput")
    cos_d = nc.dram_tensor("cosF", [T, HALF], F32, kind="ExternalInput")
    sin_d = nc.dram_tensor("sinF", [T, HALF], F32, kind="ExternalInput")
    out_d = nc.dram_tensor("out", [T, C], F32, kind="ExternalOutput")

    with ExitStack() as ctx:
        tc = ctx.enter_context(tile.TileContext(nc))
        res = ctx.enter_context(tc.tile_pool(name="res", bufs=1))
        work = ctx.enter_context(tc.tile_pool(name="work", bufs=3))
        expp = ctx.enter_context(tc.tile_pool(name="expp", bufs=2))
        outp = ctx.enter_context(tc.tile_pool(name="outp", bufs=2))
        # PSUM: big pool holds QK score batches [128,2,512]f32 (2 banks/slot)
        # and projection accumulators [128,384]f32; small pool holds PV
        # accumulators / transposes / out-proj (1 bank/slot). 3*2 + 2 = 8 banks.
        pbig = ctx.enter_context(tc.tile_pool(name="pbig", bufs=2, space="PSUM"))
        psmall = ctx.enter_context(tc.tile_pool(name="psmall", bufs=4, space="PSUM"))

        # ---- resident tiles ----
        xT_sb = res.tile([P, NCT, T], F32R)
        wq_sb = res.tile([P, NCT, C], F32R)
        wk_sb = res.tile([P, NCT, C], F32R)
        wv_sb = res.tile([P, NCT, C], F32R)
        wo_sb = res.tile([P, NCT, C], BF16)
        cos_sb = res.tile([P, NT, HALF], F32)
        sin_sb = res.tile([P, NT, HALF], F32)
        ident = res.tile([P, P], BF16)
        qT_sb = res.tile([P, NCT, T], BF16)
        kT_sb = res.tile([P, NCT, T], BF16)
        v_ext = res.tile([P, NT, H, D + 1], BF16)
        yT_sb = res.tile([P, NCT, T], BF16)

        for ci in range(NCT):
            nc.sync.dma_start(
                xT_sb[:, ci, :], xT_d[ci * P : (ci + 1) * P, :]
            )
        for w_sb, w_d in ((wq_sb, wqT_d), (wk_sb, wkT_d), (wv_sb, wvT_d), (wo_sb, woT_d)):
            for ci in range(NCT):
                nc.sync.dma_start(w_sb[:, ci, :], w_d[ci * P : (ci + 1) * P, :])
        nc.sync.dma_start(cos_sb[:], cos_d[:].rearrange("(i p) f -> p i f", p=P))
        nc.sync.dma_start(sin_sb[:], sin_d[:].rearrange("(i p) f -> p i f", p=P))
        make_identity(nc, ident[:])
        # ones column for row sums (col D of every head slot stays 1.0)
        nc.vector.memset(v_ext[:], 1.0)
        eps_sb = res.tile([P, 1], F32)
        nc.vector.memset(eps_sb[:], EPS)

        # ---- stage 1: projections (+ RoPE + RMS-norm for q,k) ----
        def proj_psum(w_sb, i):
            ps = pbig.tile([P, C], F32, tag="pbig")
            for ci in range(NCT):
                nc.tensor.matmul(
                    ps[:],
                    xT_sb[:, ci, i * P : (i + 1) * P],
                    w_sb[:, ci, :],
                    start=(ci == 0),
                    stop=(ci == NCT - 1),
                )
            return ps

        # v first: needed earliest by PV
        for i in range(NT):
            ps = proj_psum(wv_sb, i)
            nc.vector.tensor_copy(
                v_ext[:, i, :, 0:D], ps[:].rearrange("p (h d) -> p h d", h=H)
            )

        def rope_rms_transpose(w_sb, i, dstT):
            ps = proj_psum(w_sb, i)  # [P, C] f32, natural [t, (h d)]
            ph = ps[:].rearrange("p (h d) -> p h d", h=H)
            x1 = ph[:, :, 0:HALF]
            x2 = ph[:, :, HALF:D]
            cb = cos_sb[:, i, :].unsqueeze(1).broadcast_to([P, H, HALF])
            sb = sin_sb[:, i, :].unsqueeze(1).broadcast_to([P, H, HALF])

            rn = work.tile([P, H, D], F32, tag="rope_out")
            t1 = work.tile([P, H, HALF], F32, tag="rope_t1")
            t2 = work.tile([P, H, HALF], F32, tag="rope_t2")
            t3 = work.tile([P, H, HALF], F32, tag="rope_t3")
            t4 = work.tile([P, H, HALF], F32, tag="rope_t4")
            # y1 = x1*cos + x2*sin ; y2 = x2*cos - x1*sin
            nc.vector.tensor_mul(t1[:], x1, cb)
            nc.vector.tensor_mul(t2[:], x2, sb)
            nc.vector.tensor_add(rn[:, :, 0:HALF], t1[:], t2[:])
            nc.vector.tensor_mul(t3[:], x2, cb)
            nc.vector.tensor_mul(t4[:], x1, sb)
            nc.vector.tensor_sub(rn[:, :, HALF:D], t3[:], t4[:])

            # rms scale = 1/sqrt(mean(rn^2) + eps), per (t, h).
            # rsqrt = exp(-0.5*ln(.)): keeps ACT on the ln/exp table set
            # (no table switches against the softmax exps).
            sq = work.tile([P, H, D], F32, tag="sq")
            ms = work.tile([P, H], F32, tag="ms")
            lnm = work.tile([P, H], F32, tag="lnm")
            rinv = work.tile([P, H], F32, tag="rinv")
            nc.scalar.square(sq[:], rn[:])
            nc.vector.tensor_reduce(ms[:], sq[:], axis=mybir.AxisListType.X, op=ALU.add)
            nc.scalar.activation(lnm[:], ms[:], AF.Ln, bias=eps_sb[:], scale=1.0 / D)
            nc.scalar.activation(rinv[:], lnm[:], AF.Exp, scale=-0.5)

            qn = work.tile([P, C], BF16, tag="qn")
            qnh = qn[:].rearrange("p (h d) -> p h d", h=H)
            for h in range(H):
                nc.vector.tensor_scalar_mul(
                    qnh[:, h, :], rn[:, h, :], rinv[:, h : h + 1]
                )
            # transpose [t, c] -> [c, t] per c-tile (DMA cannot read PSUM)
            for ci in range(NCT):
                tp = psmall.tile([P, P], BF16, tag="psmall")
                nc.tensor.transpose(tp[:], qn[:, ci * P : (ci + 1) * P], ident[:])
                nc.vector.tensor_copy(dstT[:, ci, i * P : (i + 1) * P], tp[:])

        for i in range(NT):
            rope_rms_transpose(wk_sb, i, kT_sb)
        for i in range(NT):
            rope_rms_transpose(wq_sb, i, qT_sb)

        # ---- stage 2: attention per (head, t-block) ----
        for h in range(H):
            ci = h // 2
            r0 = (h % 2) * D  # partition offset of head h inside c-tile ci
            for tb in range(NTB):
                expS = expp.tile([P, NT, TBW], BF16, tag="expS")
                for j2 in range(NT // 2):
                    sps = pbig.tile([P, 2, TBW], F32, tag="pbig")
                    for jj in range(2):
                        j = j2 * 2 + jj
                        nc.tensor.matmul(
                            sps[:, jj, :],
                            kT_sb[r0 : r0 + D, ci, j * P : (j + 1) * P],
                            qT_sb[r0 : r0 + D, ci, tb * TBW : (tb + 1) * TBW],
                            start=True,
                            stop=True,
                        )
                    nc.scalar.activation(
                        expS[:, 2 * j2 : 2 * j2 + 2, :],
                        sps[:],
                        AF.Exp,
                        scale=1.0 / math.sqrt(D),
                    )
                # PV (swapped): yu[65, t] = sum_s [v_h | 1][s,:].T @ expS[s, t]
                # row 64 is the softmax row-sum for each t.
                yu = psmall.tile([D + 1, TBW], F32, tag="psmall")
                for j in range(NT):
                    nc.tensor.matmul(
                        yu[:],
                        v_ext[:, j, h, :],
                        expS[:, j, :],
                        start=(j == 0),
                        stop=(j == NT - 1),
                    )
                # normalize: yT = yu[0:64] * (1/rowsum) broadcast over d
                rr = work.tile([1, TBW], F32, tag="rr")
                nc.vector.reciprocal(rr[:], yu[D : D + 1, :])
                rb = work.tile([D, TBW], F32, tag="rb")
                nc.gpsimd.partition_broadcast(rb[:], rr[0:1, :])
                ci2 = h // 2
                r02 = (h % 2) * D
                nc.vector.tensor_mul(
                    yT_sb[r02 : r02 + D, ci2, tb * TBW : (tb + 1) * TBW],
                    yu[0:D, :],
                    rb[:],
                )

        # ---- stage 3: output projection from yT ----
        for i in range(NT):
            op = pbig.tile([P, C], F32, tag="pbig")
            for ci in range(NCT):
                nc.tensor.matmul(
                    op[:],
                    yT_sb[:, ci, i * P : (i + 1) * P],
                    wo_sb[:, ci, :],
                    start=(ci == 0),
                    stop=(ci == NCT - 1),
                )
            ot = outp.tile([P, C], F32, tag="ot")
            nc.vector.tensor_copy(ot[:], op[:])
            nc.sync.dma_start(out_d[i * P : (i + 1) * P, :], ot[:])

    nc.compile()
    return nc


_NC_CACHE = None


def _get_nc():
    global _NC_CACHE
    if _NC_CACHE is None:
        _NC_CACHE = build_kernel()
    return _NC_CACHE


def _in_maps(x, Wq, Wk, Wv, Wo, cos, sin):
    f32 = np.float32
    shared = {
        "wqT": np.ascontiguousarray(np.asarray(Wq, f32).T),
        "wkT": np.ascontiguousarray(np.asarray(Wk, f32).T),
        "wvT": np.ascontiguousarray(np.asarray(Wv, f32).T),
        "woT": np.ascontiguousarray(
            np.asarray(Wo, f32).T.astype(ml_dtypes.bfloat16)
        ),
        "cosF": np.ascontiguousarray(np.asarray(cos, f32).reshape(T, HALF)),
        "sinF": np.ascontiguousarray(np.asarray(sin, f32).reshape(T, HALF)),
    }
    x = np.asarray(x, f32)
    return [
        {"xT": np.ascontiguousarray(x[b].T), **shared} for b in range(NB)
    ]


def kernel(x, Wq, Wk, Wv, Wo, cos, sin, _profile=False):
    nc = _get_nc()
    in_maps = _in_maps(x, Wq, Wk, Wv, Wo, cos, sin)
    r = run_bass_kernel_spmd(nc, in_maps, core_ids=list(range(NB)), trace=_profile)
    out = np.stack([r.results[b]["out"] for b in range(NB)]).astype(np.float32)
    if _profile:
        kernel.last_results = r
    return out
